# revision 10
# baseline (speedup 1.0000x reference)
"""AttentionFreeTransformer on 8 trn2 NeuronCores.

Sharding: batch b -> core pair (2b, 2b+1); each core owns half the sequence
(T = S/2 tokens). The AFT cumsum couples the sequence dim only through the
running per-channel totals, so the pair exchanges one [D] vector per cumsum'd
tensor via a tiny pair-wise AllReduce (masked so the first half contributes
and the second half applies).

On-chip layout is channel-major [c, t] everywhere, which makes every matmul
operand load natural (weights are pre-transposed on the host) and the seq
cumsum a DVE prefix scan along the free dim. No on-chip transposes at all.

  matmul1: qkv^T[c,t] = sum_d w_qkvT[d,c] * x^T[d,t]  (rms(x) folded into the
           psum drain as a per-token scale, since rms commutes with matmul)
  middle:  rms(q), rms(k) via ones-lhsT matmuls for the cross-partition sum;
           exp/sigmoid on ACT; cumsum via tensor_tensor_scan with the
           cross-core carry as the scan's initial value.
  matmul2: uv^T[f,t] = sum_d w_swigluT[d,f] * y^T[d,t]; h^T = u*silu(g)
  matmul3: out[t,d] = sum_f h^T[f,t] * w_outT[f,d] + x  (token-major psum,
           so the residual add and the output DMA are both natural)
"""

import os
import sys

for _p in ("/opt/trn_rl_repo", "/root/.axon_site/_ro/trn_rl_repo"):
    if os.path.isdir(_p) and _p not in sys.path:
        sys.path.append(_p)

import numpy as np
import ml_dtypes

import concourse.bass as bass
import concourse.mybir as mybir
import concourse.tile as tile
from concourse import bacc
from concourse.bass_utils import run_bass_kernel_spmd

F32 = mybir.dt.float32
BF16 = mybir.dt.bfloat16
AF = mybir.ActivationFunctionType
ALU = mybir.AluOpType

EPS = 1.1920929e-07  # torch rms_norm eps=None -> finfo(float32).eps
P = 128
N_CORES = 8


def build_nc(B, S, D, DFF):
    """Build the single-core SPMD program (same on all 8 cores)."""
    assert B * 2 == N_CORES
    T = S // 2             # tokens per core
    TD = D // P            # d-chunks (contraction)
    C3 = 3 * D
    FU = DFF // P          # u f-tiles (same count for g)
    TC = min(512, T)       # token chunk for matmul free dim
    NT = T // TC           # token chunks
    KG = min(8, FU)        # matmul3 k-group size
    DC = min(512, D)       # matmul3 d-chunk
    ND = D // DC
    assert T % P == 0 and D % P == 0 and DFF % P == 0 and FU % KG == 0

    nc = bacc.Bacc("TRN2", target_bir_lowering=False, debug=False,
                   num_devices=N_CORES)

    xT_d = nc.dram_tensor("xT", [D, T], BF16, kind="ExternalInput")
    xres_d = nc.dram_tensor("xres", [T, D], F32, kind="ExternalInput")
    wq_d = nc.dram_tensor("wqkvT", [D, C3], BF16, kind="ExternalInput")
    ws_d = nc.dram_tensor("wsT", [D, 2 * DFF], BF16, kind="ExternalInput")
    wo_d = nc.dram_tensor("woT", [DFF, D], BF16, kind="ExternalInput")
    mask_d = nc.dram_tensor("mask", [1, 2], F32, kind="ExternalInput")
    out_d = nc.dram_tensor("out", [T, D], F32, kind="ExternalOutput")

    cc_in = nc.dram_tensor("cc_in", [P, 2 * TD], F32)
    cc_out = nc.dram_tensor("cc_out", [P, 2 * TD], F32)

    xT_v = xT_d.ap().rearrange("(o p) t -> p o t", p=P)      # [P, TD, T]
    wq_v = wq_d.ap().rearrange("(o p) c -> p o c", p=P)      # [P, TD, C3]
    ws_v = ws_d.ap().rearrange("(o p) f -> p o f", p=P)      # [P, TD, 2DFF]
    wo_v = wo_d.ap().rearrange("(o p) d -> p o d", p=P)      # [P, FU, D]
    xr_v = xres_d.ap().rearrange("(o p) d -> p o d", p=P)    # [P, T//P, D]
    out_v = out_d.ap().rearrange("(o p) d -> p o d", p=P)    # [P, T//P, D]

    with tile.TileContext(nc) as tc:
        persist = tc.alloc_tile_pool(name="persist", bufs=1)

        ones_col = persist.tile([P, 1], BF16, name="ones_col")
        nc.vector.memset(ones_col[:], 1.0)
        mask_rep = persist.tile([P, 2], F32, name="mask_rep")
        nc.sync.dma_start(mask_rep[:], mask_d.ap().to_broadcast((P, 2)))

        big = tc.alloc_tile_pool(name="big", bufs=1)
        poolA = tc.alloc_tile_pool(name="phaseA", bufs=1)
        psA = tc.alloc_tile_pool(name="psA", bufs=1, space="PSUM")

        def rms_rep(src_of_dk, label):
            """inv_rms over the partition axis (c/d) of a [P, TD, T] tensor,
            returned replicated to [P, T] f32. src_of_dk(dk) -> AP [P, T]."""
            rows = [psA.tile([P, TC], F32, name=f"psr_{label}_{ncb}",
                             tag="psr", bufs=2 * NT, space="PSUM")
                    for ncb in range(NT)]
            for dk in range(TD):
                sq = poolA.tile([P, T], BF16, name=f"sq_{label}_{dk}",
                                tag="sq", bufs=3)
                nc.scalar.square(sq[:], src_of_dk(dk))
                for ncb in range(NT):
                    nc.tensor.matmul(rows[ncb][0:1, :], ones_col[:],
                                     sq[:, ncb * TC:(ncb + 1) * TC],
                                     start=(dk == 0), stop=(dk == TD - 1))
            a_row = persist.tile([1, T], F32, name=f"a_{label}", tag="row",
                                 bufs=2)
            for ncb in range(NT):
                nc.vector.tensor_scalar(a_row[:, ncb * TC:(ncb + 1) * TC],
                                        rows[ncb][0:1, :], 1.0 / D, EPS,
                                        ALU.mult, ALU.add)
            s_row = persist.tile([1, T], F32, name=f"s_{label}", tag="row",
                                 bufs=2)
            nc.scalar.sqrt(s_row[:], a_row[:])
            i_row = persist.tile([1, T], F32, name=f"i_{label}", tag="row",
                                 bufs=2)
            nc.vector.reciprocal(i_row[:], s_row[:])
            rep = persist.tile([P, T], F32, name=f"rep_{label}", tag="rep",
                               bufs=2)
            nc.gpsimd.partition_broadcast(rep[:], i_row[:])
            return rep

        def big_tile(name):
            return big.tile([P, TD, T], BF16, name=name, tag="big", bufs=4)

        xT_sb = poolA.tile([P, TD, T], BF16, name="xT_sb")
        nc.sync.dma_start(xT_sb[:], xT_v[:])

        invx = rms_rep(lambda dk: xT_sb[:, dk, :], "x")

        qT = big_tile("qT")
        kT = big_tile("kT")
        vT = big_tile("vT")

        # matmul1: k tiles first, then v, then q (so the k/v paths start
        # while q is still being produced)
        order = (list(range(TD, 2 * TD)) + list(range(2 * TD, 3 * TD))
                 + list(range(0, TD)))
        for ct in order:
            wq_t = poolA.tile([P, TD, P], BF16, name=f"wq_{ct}",
                              tag="wq", bufs=3)
            nc.sync.dma_start(wq_t[:], wq_v[:, :, ct * P:(ct + 1) * P])
            for ncb in range(NT):
                ps = psA.tile([P, TC], F32, name=f"mm1_{ct}_{ncb}",
                              tag="mm1", bufs=3, space="PSUM")
                for dk in range(TD):
                    nc.tensor.matmul(
                        ps[:], wq_t[:, dk, :],
                        xT_sb[:, dk, ncb * TC:(ncb + 1) * TC],
                        start=(dk == 0), stop=(dk == TD - 1))
                grp, loc = divmod(ct, TD)
                dst = (qT, kT, vT)[grp]
                nc.vector.tensor_tensor(
                    dst[:, loc, ncb * TC:(ncb + 1) * TC], ps[:],
                    invx[:, ncb * TC:(ncb + 1) * TC], ALU.mult)

        # ---- k path: w = exp(rms(k)) ----
        invk = rms_rep(lambda dk: kT[:, dk, :], "k")
        nc.vector.tensor_tensor(
            kT[:], kT[:], invk[:, None, :].to_broadcast((P, TD, T)), ALU.mult)
        w = big_tile("w")
        nc.scalar.activation(w[:], kT[:], AF.Exp)

        # ---- q path: sig = sigmoid(rms(q)) ----
        invq = rms_rep(lambda dk: qT[:, dk, :], "q")
        nc.vector.tensor_tensor(
            qT[:], qT[:], invq[:, None, :].to_broadcast((P, TD, T)), ALU.mult)
        sig = big_tile("sig")
        nc.scalar.activation(sig[:], qT[:], AF.Sigmoid)

        # phase-A transients (xT, wq stream, sq scratch, row psums) done:
        # free their SBUF/PSUM zones for the later pools.
        poolA.release()
        psA.release()

        # ---- wv, per-channel totals, pair-wise carry exchange ----
        wv = big_tile("wv")
        nc.vector.tensor_tensor(wv[:], w[:], vT[:], ALU.mult)

        totw = persist.tile([P, 2 * TD], F32, name="totw")
        nc.vector.tensor_reduce(totw[:, 0:TD], w[:],
                                mybir.AxisListType.X, ALU.add)
        nc.vector.tensor_reduce(totw[:, TD:2 * TD], wv[:],
                                mybir.AxisListType.X, ALU.add)
        cc_sb = persist.tile([P, 2 * TD], F32, name="cc_sb")
        nc.vector.tensor_scalar_mul(cc_sb[:], totw[:], mask_rep[:, 0:1])
        nc.sync.dma_start(cc_in.ap(), cc_sb[:])
        nc.gpsimd.collective_compute(
            "AllReduce", ALU.add,
            replica_groups=[[2 * b, 2 * b + 1] for b in range(B)],
            ins=[cc_in.ap().opt()], outs=[cc_out.ap().opt()])
        carry_raw = persist.tile([P, 2 * TD], F32, name="carry_raw")
        nc.sync.dma_start(carry_raw[:], cc_out.ap())
        carry = persist.tile([P, 2 * TD], F32, name="carry")
        nc.vector.tensor_scalar_mul(carry[:], carry_raw[:], mask_rep[:, 1:2])

        # ---- cumsums along t (DVE prefix scan, carry as initial) ----
        wcum = big_tile("wcum")
        kvcum = big_tile("kvcum")
        for ct in range(TD):
            nc.vector.tensor_tensor_scan(
                wcum[:, ct], w[:, ct], w[:, ct], carry[:, ct:ct + 1],
                ALU.add, ALU.bypass)
        for ct in range(TD):
            nc.vector.tensor_tensor_scan(
                kvcum[:, ct], wv[:, ct], wv[:, ct],
                carry[:, TD + ct:TD + ct + 1], ALU.add, ALU.bypass)

        # ---- y = sig * kvcum / (wcum + 1e-6)  (produced as y^T) ----
        nc.vector.tensor_scalar_add(wcum[:], wcum[:], 1e-6)
        rcp = big_tile("rcp")
        with nc.allow_low_precision("bf16 reciprocal fine for 2e-2 gate"):
            nc.vector.reciprocal(rcp[:], wcum[:])
        nc.vector.tensor_tensor(kvcum[:], kvcum[:], rcp[:], ALU.mult)

        yT = big_tile("yT")
        nc.vector.tensor_tensor(yT[:], kvcum[:], sig[:], ALU.mult)

        # ---- matmul2 (uv^T, h^T = u*silu(g)) + matmul3 (+residual) ----
        # h^T is held as two half tiles so it can recycle "big"-tag slots.
        FH = FU // 2
        poolB = tc.alloc_tile_pool(name="phaseB", bufs=1)
        psB = tc.alloc_tile_pool(name="psB", bufs=1, space="PSUM")
        for tci in range(NT):
            tsl = slice(tci * TC, (tci + 1) * TC)
            hT_halves = [big.tile([P, FH, TC], BF16, name=f"hT_{tci}_{i}",
                                  tag="big", bufs=4) for i in range(2)]

            def hT_slice(k, tt):
                return hT_halves[k // FH][:, k % FH, tt * P:(tt + 1) * P]

            for fj in range(FU):
                wu_t = poolB.tile([P, TD, P], BF16, name=f"wu_{tci}_{fj}",
                                  tag="ws", bufs=3)
                nc.sync.dma_start(wu_t[:], ws_v[:, :, fj * P:(fj + 1) * P])
                wg_t = poolB.tile([P, TD, P], BF16, name=f"wg_{tci}_{fj}",
                                  tag="ws", bufs=3)
                nc.sync.dma_start(
                    wg_t[:], ws_v[:, :, DFF + fj * P:DFF + (fj + 1) * P])
                psu = psB.tile([P, TC], F32, name=f"psu_{tci}_{fj}",
                               tag="mm2", bufs=4, space="PSUM")
                psg = psB.tile([P, TC], F32, name=f"psg_{tci}_{fj}",
                               tag="mm2", bufs=4, space="PSUM")
                for dk in range(TD):
                    nc.tensor.matmul(psu[:], wu_t[:, dk, :], yT[:, dk, tsl],
                                     start=(dk == 0), stop=(dk == TD - 1))
                for dk in range(TD):
                    nc.tensor.matmul(psg[:], wg_t[:, dk, :], yT[:, dk, tsl],
                                     start=(dk == 0), stop=(dk == TD - 1))
                sg = poolB.tile([P, TC], BF16, name=f"sg_{tci}_{fj}",
                                tag="sg", bufs=3)
                nc.scalar.activation(sg[:], psg[:], AF.Sigmoid)
                gsg = poolB.tile([P, TC], BF16, name=f"gsg_{tci}_{fj}",
                                 tag="sg", bufs=3)
                nc.vector.tensor_tensor(gsg[:], psg[:], sg[:], ALU.mult)
                nc.vector.tensor_tensor(hT_halves[fj // FH][:, fj % FH, :],
                                        psu[:], gsg[:], ALU.mult)

            n_tt = TC // P
            for dc in range(ND):
                dsl = slice(dc * DC, (dc + 1) * DC)
                ps3 = [psB.tile([P, DC], F32, name=f"ps3_{tci}_{dc}_{tt}",
                                tag="mm3", bufs=4, space="PSUM")
                       for tt in range(n_tt)]
                for kg in range(FU // KG):
                    wo_t = poolB.tile([P, KG, DC], BF16,
                                      name=f"wo_{tci}_{dc}_{kg}",
                                      tag="wo", bufs=2)
                    nc.sync.dma_start(
                        wo_t[:], wo_v[:, kg * KG:(kg + 1) * KG, dsl])
                    for tt in range(n_tt):
                        for kk in range(KG):
                            k = kg * KG + kk
                            nc.tensor.matmul(
                                ps3[tt][:], hT_slice(k, tt), wo_t[:, kk, :],
                                start=(k == 0), stop=(k == FU - 1))
                for tt in range(n_tt):
                    tt_g = tci * (TC // P) + tt
                    xr_t = poolB.tile([P, DC], F32,
                                      name=f"xr_{tci}_{dc}_{tt}",
                                      tag="xr", bufs=3)
                    nc.sync.dma_start(xr_t[:], xr_v[:, tt_g, dsl])
                    o_t = poolB.tile([P, DC], F32, name=f"o_{tci}_{dc}_{tt}",
                                     tag="ot", bufs=3)
                    nc.vector.tensor_tensor(o_t[:], ps3[tt][:], xr_t[:],
                                            ALU.add)
                    nc.sync.dma_start(out_v[:, tt_g, dsl], o_t[:])

        poolB.release()
        psB.release()
        big.release()
        persist.release()

    nc.compile()
    return nc


_NC_CACHE = {}


def _get_nc(B, S, D, DFF):
    key = (B, S, D, DFF)
    if key not in _NC_CACHE:
        _NC_CACHE[key] = build_nc(B, S, D, DFF)
    return _NC_CACHE[key]


def make_in_maps(x, w_qkv, w_swiglu, w_out):
    B, S, D = x.shape
    T = S // 2
    bf = ml_dtypes.bfloat16
    wq_T = w_qkv.T.astype(bf)
    ws_T = w_swiglu.T.astype(bf)
    wo_T = w_out.T.astype(bf)
    in_maps = []
    for c in range(N_CORES):
        b, h = divmod(c, 2)
        xc = x[b, h * T:(h + 1) * T]
        in_maps.append({
            "xT": xc.T.astype(bf),
            "xres": np.ascontiguousarray(xc, dtype=np.float32),
            "wqkvT": wq_T,
            "wsT": ws_T,
            "woT": wo_T,
            "mask": np.array([[1.0 - h, float(h)]], np.float32),
        })
    return in_maps


def assemble_out(results, B, S, D):
    T = S // 2
    out = np.empty((B, S, D), np.float32)
    for c in range(N_CORES):
        b, h = divmod(c, 2)
        out[b, h * T:(h + 1) * T] = results[c]["out"]
    return out


def kernel(x, w_qkv, w_swiglu, w_out):
    x = np.asarray(x, dtype=np.float32)
    w_qkv = np.asarray(w_qkv, dtype=np.float32)
    w_swiglu = np.asarray(w_swiglu, dtype=np.float32)
    w_out = np.asarray(w_out, dtype=np.float32)
    B, S, D = x.shape
    DFF = w_out.shape[1]
    nc = _get_nc(B, S, D, DFF)
    in_maps = make_in_maps(x, w_qkv, w_swiglu, w_out)
    res = run_bass_kernel_spmd(nc, in_maps, core_ids=list(range(N_CORES)))
    return assemble_out(res.results, B, S, D)


# revision 18
# speedup vs baseline: 1.1986x; 1.1986x over previous
"""AttentionFreeTransformer on 8 trn2 NeuronCores.

Sharding: batch b -> core pair (2b, 2b+1); each core owns half the sequence
(T = S/2 tokens). The AFT cumsum couples the sequence dim only through the
running per-channel totals, so the pair exchanges one [D] vector per cumsum'd
tensor via a tiny pair-wise AllReduce (masked so the first half contributes
and the second half applies).

On-chip layout is channel-major [c, t] everywhere, which makes every matmul
operand load natural (weights are pre-transposed on the host) and the seq
cumsum a DVE prefix scan along the free dim. No on-chip transposes at all.

  matmul1: qkv^T[c,t] = sum_d w_qkvT[d,c] * x^T[d,t]  (rms(x) folded into the
           psum drain as a per-token scale, since rms commutes with matmul)
  middle:  rms(q), rms(k) via ones-lhsT matmuls for the cross-partition sum;
           exp/sigmoid on ACT; cumsum via tensor_tensor_scan with the
           cross-core carry as the scan's initial value.
  matmul2: uv^T[f,t] = sum_d w_swigluT[d,f] * y^T[d,t]; h^T = u*silu(g)
  matmul3: out[t,d] = sum_f h^T[f,t] * w_outT[f,d] + x  (token-major psum,
           so the residual add and the output DMA are both natural)
"""

import os
import sys

for _p in ("/opt/trn_rl_repo", "/root/.axon_site/_ro/trn_rl_repo"):
    if os.path.isdir(_p) and _p not in sys.path:
        sys.path.append(_p)

import numpy as np
import ml_dtypes

import concourse.bass as bass
import concourse.mybir as mybir
import concourse.tile as tile
from concourse import bacc
from concourse.bass_utils import run_bass_kernel_spmd

F32 = mybir.dt.float32
BF16 = mybir.dt.bfloat16
AF = mybir.ActivationFunctionType
ALU = mybir.AluOpType

EPS = 1.1920929e-07  # torch rms_norm eps=None -> finfo(float32).eps
P = 128
N_CORES = 8


def build_nc(B, S, D, DFF):
    """Build the single-core SPMD program (same on all 8 cores)."""
    assert B * 2 == N_CORES
    T = S // 2             # tokens per core
    TD = D // P            # d-chunks (contraction)
    C3 = 3 * D
    FU = DFF // P          # u f-tiles (same count for g)
    TC = min(512, T)       # token chunk for matmul free dim
    NT = T // TC           # token chunks
    KG = min(8, FU)        # matmul3 k-group size
    DC = min(512, D)       # matmul3 d-chunk
    ND = D // DC
    assert T % P == 0 and D % P == 0 and DFF % P == 0 and FU % KG == 0

    nc = bacc.Bacc("TRN2", target_bir_lowering=False, debug=False,
                   num_devices=N_CORES)

    xT_d = nc.dram_tensor("xT", [D, T], BF16, kind="ExternalInput")
    xres_d = nc.dram_tensor("xres", [T, D], F32, kind="ExternalInput")
    wq_d = nc.dram_tensor("wqkvT", [D, C3], BF16, kind="ExternalInput")
    ws_d = nc.dram_tensor("wsT", [D, 2 * DFF], BF16, kind="ExternalInput")
    wo_d = nc.dram_tensor("woT", [DFF, D], BF16, kind="ExternalInput")
    mask_d = nc.dram_tensor("mask", [1, 2], F32, kind="ExternalInput")
    out_d = nc.dram_tensor("out", [T, D], F32, kind="ExternalOutput")

    cc_in = nc.dram_tensor("cc_in", [P, 2 * TD], F32)
    cc_out = nc.dram_tensor("cc_out", [P, 2 * TD], F32)

    xT_v = xT_d.ap().rearrange("(o p) t -> p o t", p=P)      # [P, TD, T]
    wq_v = wq_d.ap().rearrange("(o p) c -> p o c", p=P)      # [P, TD, C3]
    ws_v = ws_d.ap().rearrange("(o p) f -> p o f", p=P)      # [P, TD, 2DFF]
    wo_v = wo_d.ap().rearrange("(o p) d -> p o d", p=P)      # [P, FU, D]
    xr_v = xres_d.ap().rearrange("(o p) d -> p o d", p=P)    # [P, T//P, D]
    out_v = out_d.ap().rearrange("(o p) d -> p o d", p=P)    # [P, T//P, D]

    with tile.TileContext(nc) as tc:
        persist = tc.alloc_tile_pool(name="persist", bufs=1)

        ones_col = persist.tile([P, 1], BF16, name="ones_col")
        nc.vector.memset(ones_col[:], 1.0)
        mask_rep = persist.tile([P, 2], F32, name="mask_rep")
        nc.sync.dma_start(mask_rep[:], mask_d.ap().to_broadcast((P, 2)))

        big = tc.alloc_tile_pool(name="big", bufs=1)
        small = tc.alloc_tile_pool(name="small", bufs=1)
        poolA = tc.alloc_tile_pool(name="phaseA", bufs=1)
        psA = tc.alloc_tile_pool(name="psA", bufs=1, space="PSUM")

        def rms_rep(src_of_dk, label, rep_dtype=F32):
            """inv_rms over the partition axis (c/d) of a [P, TD, T] tensor,
            returned replicated to [P, T]. src_of_dk(dk) -> AP [P, T]."""
            rows = [psA.tile([P, TC], F32, name=f"psr_{label}_{ncb}",
                             tag="psr", bufs=2 * NT, space="PSUM")
                    for ncb in range(NT)]
            for dk in range(TD):
                sq = poolA.tile([P, T], BF16, name=f"sq_{label}_{dk}",
                                tag="sq", bufs=3)
                nc.scalar.square(sq[:], src_of_dk(dk))
                for ncb in range(NT):
                    nc.tensor.matmul(rows[ncb][0:1, :], ones_col[:],
                                     sq[:, ncb * TC:(ncb + 1) * TC],
                                     start=(dk == 0), stop=(dk == TD - 1))
            a_row = persist.tile([1, T], F32, name=f"a_{label}", tag="row",
                                 bufs=2)
            for ncb in range(NT):
                nc.vector.tensor_scalar(a_row[:, ncb * TC:(ncb + 1) * TC],
                                        rows[ncb][0:1, :], 1.0 / D, EPS,
                                        ALU.mult, ALU.add)
            s_row = persist.tile([1, T], F32, name=f"s_{label}", tag="row",
                                 bufs=2)
            nc.scalar.sqrt(s_row[:], a_row[:])
            i_row = persist.tile([1, T], F32, name=f"i_{label}", tag="row",
                                 bufs=2)
            nc.vector.reciprocal_approx_fast(i_row[:], s_row[:])
            if rep_dtype != F32:
                ib_row = persist.tile([1, T], rep_dtype, name=f"ib_{label}",
                                      tag="rowb", bufs=2)
                nc.vector.tensor_copy(ib_row[:], i_row[:])
                i_row = ib_row
            rep = persist.tile([P, T], rep_dtype, name=f"rep_{label}",
                               tag=f"rep_{rep_dtype}",
                               bufs=(1 if rep_dtype == F32 else 2))
            nc.gpsimd.partition_broadcast(rep[:], i_row[:])
            return rep

        def big_tile(name):
            return big.tile([P, TD, T], BF16, name=name, tag="big", bufs=4)

        xT_sb = poolA.tile([P, TD, T], BF16, name="xT_sb")
        nc.sync.dma_start(xT_sb[:], xT_v[:])

        invx = rms_rep(lambda dk: xT_sb[:, dk, :], "x")

        qT = big_tile("qT")
        kT = big_tile("kT")
        vT = big_tile("vT")

        def mm1_tiles(cts):
            for ct in cts:
                wq_t = poolA.tile([P, TD, P], BF16, name=f"wq_{ct}",
                                  tag="wq", bufs=2)
                nc.sync.dma_start(wq_t[:], wq_v[:, :, ct * P:(ct + 1) * P])
                for ncb in range(NT):
                    ps = psA.tile([P, TC], F32, name=f"mm1_{ct}_{ncb}",
                                  tag="mm1", bufs=3, space="PSUM")
                    for dk in range(TD):
                        nc.tensor.matmul(
                            ps[:], wq_t[:, dk, :],
                            xT_sb[:, dk, ncb * TC:(ncb + 1) * TC],
                            start=(dk == 0), stop=(dk == TD - 1))
                    grp, loc = divmod(ct, TD)
                    dst = (qT, kT, vT)[grp]
                    nc.vector.tensor_tensor(
                        dst[:, loc, ncb * TC:(ncb + 1) * TC], ps[:],
                        invx[:, ncb * TC:(ncb + 1) * TC], ALU.mult)

        # matmul1 K tiles, then the k path (its ssq row-matmuls slot into the
        # PE stream here, so the exp/wv/collective chain overlaps the V and Q
        # matmul tiles below)
        mm1_tiles(range(TD, 2 * TD))
        invk = rms_rep(lambda dk: kT[:, dk, :], "k", BF16)
        nc.vector.tensor_tensor(
            kT[:], kT[:], invk[:, None, :].to_broadcast((P, TD, T)), ALU.mult)
        w = big_tile("w")
        nc.scalar.activation(w[:], kT[:], AF.Exp)

        # matmul1 V tiles, then wv + per-channel totals + carry exchange
        mm1_tiles(range(2 * TD, 3 * TD))
        wv = big_tile("wv")
        nc.vector.tensor_tensor(wv[:], w[:], vT[:], ALU.mult)

        totw = persist.tile([P, 2 * TD], F32, name="totw")
        nc.vector.tensor_reduce(totw[:, 0:TD], w[:],
                                mybir.AxisListType.X, ALU.add)
        nc.vector.tensor_reduce(totw[:, TD:2 * TD], wv[:],
                                mybir.AxisListType.X, ALU.add)
        cc_sb = persist.tile([P, 2 * TD], F32, name="cc_sb")
        nc.vector.tensor_scalar_mul(cc_sb[:], totw[:], mask_rep[:, 0:1])
        nc.sync.dma_start(cc_in.ap(), cc_sb[:])
        nc.gpsimd.collective_compute(
            "AllReduce", ALU.add,
            replica_groups=[[2 * b, 2 * b + 1] for b in range(B)],
            ins=[cc_in.ap().opt()], outs=[cc_out.ap().opt()])
        carry_raw = persist.tile([P, 2 * TD], F32, name="carry_raw")
        nc.sync.dma_start(carry_raw[:], cc_out.ap())
        carry = persist.tile([P, 2 * TD], F32, name="carry")
        nc.vector.tensor_scalar_mul(carry[:], carry_raw[:], mask_rep[:, 1:2])
        # fold the +1e-6 denominator guard into the w-scan's initial value
        carry_eps = persist.tile([P, TD], F32, name="carry_eps")
        nc.vector.tensor_scalar_add(carry_eps[:], carry[:, 0:TD], 1e-6)

        # matmul1 Q tiles, then the q path
        mm1_tiles(range(0, TD))
        invq = rms_rep(lambda dk: qT[:, dk, :], "q", BF16)
        nc.vector.tensor_tensor(
            qT[:], qT[:], invq[:, None, :].to_broadcast((P, TD, T)), ALU.mult)
        sig = big_tile("sig")
        nc.scalar.activation(sig[:], qT[:], AF.Sigmoid)

        poolA.release()
        psA.release()

        # ---- per-(tchunk, c-tile) cumsum scans -> y = sig*kvcum/(wcum+eps),
        # in small f32 scratch so matmul2 can chase y tchunk by tchunk ----
        lastw = persist.tile([P, TD], F32, name="lastw")
        lastkv = persist.tile([P, TD], F32, name="lastkv")
        y = big_tile("y")
        for tci in range(NT):
            tsl = slice(tci * TC, (tci + 1) * TC)
            for ct in range(TD):
                wc = small.tile([P, TC], F32, name=f"wc_{tci}_{ct}",
                                tag="mid", bufs=6)
                init_w = (carry_eps[:, ct:ct + 1] if tci == 0
                          else lastw[:, ct:ct + 1])
                nc.vector.tensor_tensor_scan(
                    wc[:], w[:, ct, tsl], w[:, ct, tsl], init_w,
                    ALU.add, ALU.bypass)
                kv = small.tile([P, TC], F32, name=f"kv_{tci}_{ct}",
                                tag="mid", bufs=6)
                init_kv = (carry[:, TD + ct:TD + ct + 1] if tci == 0
                           else lastkv[:, ct:ct + 1])
                nc.vector.tensor_tensor_scan(
                    kv[:], wv[:, ct, tsl], wv[:, ct, tsl], init_kv,
                    ALU.add, ALU.bypass)
                if tci + 1 < NT:
                    nc.vector.tensor_copy(lastw[:, ct:ct + 1],
                                          wc[:, TC - 1:TC])
                    nc.vector.tensor_copy(lastkv[:, ct:ct + 1],
                                          kv[:, TC - 1:TC])
                rcp = small.tile([P, TC], F32, name=f"rcp_{tci}_{ct}",
                                 tag="mid", bufs=6)
                nc.vector.reciprocal_approx_fast(rcp[:], wc[:])
                kvy = small.tile([P, TC], F32, name=f"kvy_{tci}_{ct}",
                                 tag="mid", bufs=6)
                nc.vector.tensor_tensor(kvy[:], kv[:], rcp[:], ALU.mult)
                nc.vector.tensor_tensor(y[:, ct, tsl], kvy[:],
                                        sig[:, ct, tsl], ALU.mult)
        yT = y

        small.release()

        # ---- matmul2 (uv^T, h^T = u*silu(g)) + matmul3 (+residual) ----
        # h^T is held as two half tiles so it can recycle "big"-tag slots.
        FH = FU // 2
        poolB = tc.alloc_tile_pool(name="phaseB", bufs=1)
        psB = tc.alloc_tile_pool(name="psB", bufs=1, space="PSUM")
        for tci in range(NT):
            tsl = slice(tci * TC, (tci + 1) * TC)
            hT_halves = [big.tile([P, FH, TC], BF16, name=f"hT_{tci}_{i}",
                                  tag="big", bufs=4) for i in range(2)]

            def hT_slice(k, tt):
                return hT_halves[k // FH][:, k % FH, tt * P:(tt + 1) * P]

            for fj in range(FU):
                wu_t = poolB.tile([P, TD, P], BF16, name=f"wu_{tci}_{fj}",
                                  tag="ws", bufs=4)
                nc.sync.dma_start(wu_t[:], ws_v[:, :, fj * P:(fj + 1) * P])
                wg_t = poolB.tile([P, TD, P], BF16, name=f"wg_{tci}_{fj}",
                                  tag="ws", bufs=4)
                nc.sync.dma_start(
                    wg_t[:], ws_v[:, :, DFF + fj * P:DFF + (fj + 1) * P])
                psu = psB.tile([P, TC], F32, name=f"psu_{tci}_{fj}",
                               tag="mm2", bufs=4, space="PSUM")
                psg = psB.tile([P, TC], F32, name=f"psg_{tci}_{fj}",
                               tag="mm2", bufs=4, space="PSUM")
                for dk in range(TD):
                    nc.tensor.matmul(psu[:], wu_t[:, dk, :], yT[:, dk, tsl],
                                     start=(dk == 0), stop=(dk == TD - 1))
                for dk in range(TD):
                    nc.tensor.matmul(psg[:], wg_t[:, dk, :], yT[:, dk, tsl],
                                     start=(dk == 0), stop=(dk == TD - 1))
                sg = poolB.tile([P, TC], BF16, name=f"sg_{tci}_{fj}",
                                tag="sg", bufs=4)
                nc.scalar.activation(sg[:], psg[:], AF.Sigmoid)
                gsg = poolB.tile([P, TC], BF16, name=f"gsg_{tci}_{fj}",
                                 tag="sg", bufs=4)
                nc.vector.tensor_tensor(gsg[:], psg[:], sg[:], ALU.mult)
                nc.vector.tensor_tensor(hT_halves[fj // FH][:, fj % FH, :],
                                        psu[:], gsg[:], ALU.mult)

            n_tt = TC // P
            for dc in range(ND):
                dsl = slice(dc * DC, (dc + 1) * DC)
                ps3 = [psB.tile([P, DC], F32, name=f"ps3_{tci}_{dc}_{tt}",
                                tag="mm3", bufs=4, space="PSUM")
                       for tt in range(n_tt)]
                for kg in range(FU // KG):
                    wo_t = poolB.tile([P, KG, DC], BF16,
                                      name=f"wo_{tci}_{dc}_{kg}",
                                      tag="wo", bufs=2)
                    nc.sync.dma_start(
                        wo_t[:], wo_v[:, kg * KG:(kg + 1) * KG, dsl])
                    for tt in range(n_tt):
                        for kk in range(KG):
                            k = kg * KG + kk
                            nc.tensor.matmul(
                                ps3[tt][:], hT_slice(k, tt), wo_t[:, kk, :],
                                start=(k == 0), stop=(k == FU - 1))
                for tt in range(n_tt):
                    tt_g = tci * (TC // P) + tt
                    xr_t = poolB.tile([P, DC], F32,
                                      name=f"xr_{tci}_{dc}_{tt}",
                                      tag="xr", bufs=3)
                    nc.sync.dma_start(xr_t[:], xr_v[:, tt_g, dsl])
                    o_t = poolB.tile([P, DC], F32, name=f"o_{tci}_{dc}_{tt}",
                                     tag="ot", bufs=3)
                    nc.vector.tensor_tensor(o_t[:], ps3[tt][:], xr_t[:],
                                            ALU.add)
                    nc.sync.dma_start(out_v[:, tt_g, dsl], o_t[:])

        poolB.release()
        psB.release()
        big.release()
        persist.release()

    nc.compile()
    return nc


_NC_CACHE = {}


def _get_nc(B, S, D, DFF):
    key = (B, S, D, DFF)
    if key not in _NC_CACHE:
        _NC_CACHE[key] = build_nc(B, S, D, DFF)
    return _NC_CACHE[key]


def make_in_maps(x, w_qkv, w_swiglu, w_out):
    B, S, D = x.shape
    T = S // 2
    bf = ml_dtypes.bfloat16
    wq_T = w_qkv.T.astype(bf)
    ws_T = w_swiglu.T.astype(bf)
    wo_T = w_out.T.astype(bf)
    in_maps = []
    for c in range(N_CORES):
        b, h = divmod(c, 2)
        xc = x[b, h * T:(h + 1) * T]
        in_maps.append({
            "xT": xc.T.astype(bf),
            "xres": np.ascontiguousarray(xc, dtype=np.float32),
            "wqkvT": wq_T,
            "wsT": ws_T,
            "woT": wo_T,
            "mask": np.array([[1.0 - h, float(h)]], np.float32),
        })
    return in_maps


def assemble_out(results, B, S, D):
    T = S // 2
    out = np.empty((B, S, D), np.float32)
    for c in range(N_CORES):
        b, h = divmod(c, 2)
        out[b, h * T:(h + 1) * T] = results[c]["out"]
    return out


def kernel(x, w_qkv, w_swiglu, w_out):
    x = np.asarray(x, dtype=np.float32)
    w_qkv = np.asarray(w_qkv, dtype=np.float32)
    w_swiglu = np.asarray(w_swiglu, dtype=np.float32)
    w_out = np.asarray(w_out, dtype=np.float32)
    B, S, D = x.shape
    DFF = w_out.shape[1]
    nc = _get_nc(B, S, D, DFF)
    in_maps = make_in_maps(x, w_qkv, w_swiglu, w_out)
    res = run_bass_kernel_spmd(nc, in_maps, core_ids=list(range(N_CORES)))
    return assemble_out(res.results, B, S, D)


# revision 26
# speedup vs baseline: 1.2315x; 1.0275x over previous
"""AttentionFreeTransformer on 8 trn2 NeuronCores.

Sharding: batch b -> core pair (2b, 2b+1); each core owns half the sequence
(T = S/2 tokens). The AFT cumsum couples the sequence dim only through the
running per-channel totals, so the pair exchanges one [D] vector per cumsum'd
tensor via a tiny pair-wise AllReduce (masked so the first half contributes
and the second half applies).

On-chip layout is channel-major [c, t] everywhere, which makes every matmul
operand load natural (weights are pre-transposed on the host) and the seq
cumsum a DVE prefix scan along the free dim. No on-chip transposes at all.

  matmul1: qkv^T[c,t] = sum_d w_qkvT[d,c] * x^T[d,t]  (rms(x) folded into the
           psum drain as a per-token scale, since rms commutes with matmul)
  middle:  rms(q), rms(k) via ones-lhsT matmuls for the cross-partition sum;
           exp/sigmoid on ACT; cumsum via tensor_tensor_scan with the
           cross-core carry as the scan's initial value.
  matmul2: uv^T[f,t] = sum_d w_swigluT[d,f] * y^T[d,t]; h^T = u*silu(g)
  matmul3: out[t,d] = sum_f h^T[f,t] * w_outT[f,d] + x  (token-major psum,
           so the residual add and the output DMA are both natural)
"""

import os
import sys

for _p in ("/opt/trn_rl_repo", "/root/.axon_site/_ro/trn_rl_repo"):
    if os.path.isdir(_p) and _p not in sys.path:
        sys.path.append(_p)

import numpy as np
import ml_dtypes

import concourse.bass as bass
import concourse.mybir as mybir
import concourse.tile as tile
from concourse import bacc
from concourse.bass_utils import run_bass_kernel_spmd

F32 = mybir.dt.float32
BF16 = mybir.dt.bfloat16
AF = mybir.ActivationFunctionType
ALU = mybir.AluOpType

EPS = 1.1920929e-07  # torch rms_norm eps=None -> finfo(float32).eps
P = 128
N_CORES = 8


def build_nc(B, S, D, DFF):
    """Build the single-core SPMD program (same on all 8 cores)."""
    assert B * 2 == N_CORES
    T = S // 2             # tokens per core
    TD = D // P            # d-chunks (contraction)
    C3 = 3 * D
    FU = DFF // P          # u f-tiles (same count for g)
    TC = min(512, T)       # token chunk for matmul free dim
    NT = T // TC           # token chunks
    KG = min(8, FU)        # matmul3 k-group size
    DC = min(512, D)       # matmul3 d-chunk
    ND = D // DC
    assert T % P == 0 and D % P == 0 and DFF % P == 0 and FU % KG == 0

    nc = bacc.Bacc("TRN2", target_bir_lowering=False, debug=False,
                   num_devices=N_CORES)

    xT_d = nc.dram_tensor("xT", [D, T], BF16, kind="ExternalInput")
    xres_d = nc.dram_tensor("xres", [T, D], F32, kind="ExternalInput")
    wq_d = nc.dram_tensor("wqkvT", [D, C3], BF16, kind="ExternalInput")
    ws_d = nc.dram_tensor("wsT", [D, 2 * DFF], BF16, kind="ExternalInput")
    wo_d = nc.dram_tensor("woT", [DFF, D], BF16, kind="ExternalInput")
    mask_d = nc.dram_tensor("mask", [1, 2], F32, kind="ExternalInput")
    out_d = nc.dram_tensor("out", [T, D], F32, kind="ExternalOutput")

    cc_in = nc.dram_tensor("cc_in", [P, 2 * TD], F32)
    cc_out = nc.dram_tensor("cc_out", [P, 2 * TD], F32)

    xT_v = xT_d.ap().rearrange("(o p) t -> p o t", p=P)      # [P, TD, T]
    wq_v = wq_d.ap().rearrange("(o p) c -> p o c", p=P)      # [P, TD, C3]
    ws_v = ws_d.ap().rearrange("(o p) f -> p o f", p=P)      # [P, TD, 2DFF]
    wo_v = wo_d.ap().rearrange("(o p) d -> p o d", p=P)      # [P, FU, D]
    xr_v = xres_d.ap().rearrange("(o p) d -> p o d", p=P)    # [P, T//P, D]
    out_v = out_d.ap().rearrange("(o p) d -> p o d", p=P)    # [P, T//P, D]

    with tile.TileContext(nc) as tc:
        persist = tc.alloc_tile_pool(name="persist", bufs=1)

        ones_col = persist.tile([P, 1], BF16, name="ones_col")
        nc.vector.memset(ones_col[:], 1.0)
        mask_rep = persist.tile([P, 2], F32, name="mask_rep")
        nc.sync.dma_start(mask_rep[:], mask_d.ap().to_broadcast((P, 2)))

        big = tc.alloc_tile_pool(name="big", bufs=1)
        small = tc.alloc_tile_pool(name="small", bufs=1)
        poolA = tc.alloc_tile_pool(name="phaseA", bufs=1)
        psA = tc.alloc_tile_pool(name="psA", bufs=1, space="PSUM")

        def rms_rep(src_of_dk, label, rep_dtype=F32):
            """inv_rms over the partition axis (c/d) of a [P, TD, T] tensor,
            returned replicated to [P, T]. src_of_dk(dk) -> AP [P, T]."""
            rows = [psA.tile([P, TC], F32, name=f"psr_{label}_{ncb}",
                             tag="psr", bufs=2 * NT, space="PSUM")
                    for ncb in range(NT)]
            for dk in range(TD):
                sq = poolA.tile([P, T], BF16, name=f"sq_{label}_{dk}",
                                tag="sq", bufs=3)
                nc.scalar.square(sq[:], src_of_dk(dk))
                for ncb in range(NT):
                    nc.tensor.matmul(rows[ncb][0:1, :], ones_col[:],
                                     sq[:, ncb * TC:(ncb + 1) * TC],
                                     start=(dk == 0), stop=(dk == TD - 1))
            a_row = persist.tile([1, T], F32, name=f"a_{label}", tag="row",
                                 bufs=2)
            for ncb in range(NT):
                nc.vector.tensor_scalar(a_row[:, ncb * TC:(ncb + 1) * TC],
                                        rows[ncb][0:1, :], 1.0 / D, EPS,
                                        ALU.mult, ALU.add)
            s_row = persist.tile([1, T], F32, name=f"s_{label}", tag="row",
                                 bufs=2)
            nc.scalar.sqrt(s_row[:], a_row[:])
            i_row = persist.tile([1, T], F32, name=f"i_{label}", tag="row",
                                 bufs=2)
            nc.vector.reciprocal_approx_fast(i_row[:], s_row[:])
            if rep_dtype != F32:
                ib_row = persist.tile([1, T], rep_dtype, name=f"ib_{label}",
                                      tag="rowb", bufs=2)
                nc.vector.tensor_copy(ib_row[:], i_row[:])
                i_row = ib_row
            rep = persist.tile([P, T], rep_dtype, name=f"rep_{label}",
                               tag=f"rep_{rep_dtype}",
                               bufs=(1 if rep_dtype == F32 else 2))
            nc.gpsimd.partition_broadcast(rep[:], i_row[:])
            return rep

        def big_tile(name):
            return big.tile([P, TD, T], BF16, name=name, tag="big", bufs=4)

        xT_sb = poolA.tile([P, TD, T], BF16, name="xT_sb")
        nc.sync.dma_start(xT_sb[:], xT_v[:])

        invx = rms_rep(lambda dk: xT_sb[:, dk, :], "x")

        qT = big_tile("qT")
        kT = big_tile("kT")
        vT = big_tile("vT")

        def mm1_tiles(cts, post_cb=None):
            for ct in cts:
                wq_t = poolA.tile([P, TD, P], BF16, name=f"wq_{ct}",
                                  tag="wq", bufs=2)
                nc.sync.dma_start(wq_t[:], wq_v[:, :, ct * P:(ct + 1) * P])
                for ncb in range(NT):
                    ps = psA.tile([P, TC], F32, name=f"mm1_{ct}_{ncb}",
                                  tag="mm1", bufs=3, space="PSUM")
                    for dk in range(TD):
                        nc.tensor.matmul(
                            ps[:], wq_t[:, dk, :],
                            xT_sb[:, dk, ncb * TC:(ncb + 1) * TC],
                            start=(dk == 0), stop=(dk == TD - 1))
                    grp, loc = divmod(ct, TD)
                    dst = (qT, kT, vT)[grp]
                    nc.vector.tensor_tensor(
                        dst[:, loc, ncb * TC:(ncb + 1) * TC], ps[:],
                        invx[:, ncb * TC:(ncb + 1) * TC], ALU.mult)
                if post_cb is not None:
                    post_cb(ct)

        # matmul1 K tiles, then the k path (its ssq row-matmuls slot into the
        # PE stream here, so the exp/wv/collective chain overlaps the V and Q
        # matmul tiles below)
        mm1_tiles(range(TD, 2 * TD))
        invk = rms_rep(lambda dk: kT[:, dk, :], "k", BF16)
        nc.vector.tensor_tensor(
            kT[:], kT[:], invk[:, None, :].to_broadcast((P, TD, T)), ALU.mult)
        w = big_tile("w")
        nc.scalar.activation(w[:], kT[:], AF.Exp)

        # matmul1 V tiles; wv + per-channel totals interleave per c-tile so
        # the DVE work hides under the V matmuls and the collective can fire
        # before matmul1 finishes
        wv = big_tile("wv")
        totw = persist.tile([P, 2 * TD], F32, name="totw")

        def v_post(ct):
            cl = ct - 2 * TD
            nc.vector.tensor_tensor(wv[:, cl, :], w[:, cl, :], vT[:, cl, :],
                                    ALU.mult)
            nc.vector.tensor_reduce(totw[:, cl:cl + 1], w[:, cl, :],
                                    mybir.AxisListType.X, ALU.add)
            nc.vector.tensor_reduce(totw[:, TD + cl:TD + cl + 1],
                                    wv[:, cl, :],
                                    mybir.AxisListType.X, ALU.add)

        mm1_tiles(range(2 * TD, 3 * TD), post_cb=v_post)

        cc_sb = persist.tile([P, 2 * TD], F32, name="cc_sb")
        nc.vector.tensor_scalar_mul(cc_sb[:], totw[:], mask_rep[:, 0:1])
        nc.sync.dma_start(cc_in.ap(), cc_sb[:])
        nc.gpsimd.collective_compute(
            "AllReduce", ALU.add,
            replica_groups=[[2 * b, 2 * b + 1] for b in range(B)],
            ins=[cc_in.ap().opt()], outs=[cc_out.ap().opt()])
        carry_raw = persist.tile([P, 2 * TD], F32, name="carry_raw")
        nc.sync.dma_start(carry_raw[:], cc_out.ap())
        carry = persist.tile([P, 2 * TD], F32, name="carry")
        nc.vector.tensor_scalar_mul(carry[:], carry_raw[:], mask_rep[:, 1:2])
        # fold the +1e-6 denominator guard into the w-scan's initial value
        carry_eps = persist.tile([P, TD], F32, name="carry_eps")
        nc.vector.tensor_scalar_add(carry_eps[:], carry[:, 0:TD], 1e-6)

        # matmul1 Q tiles, then the q path (sigmoid in place: sig aliases qT)
        mm1_tiles(range(0, TD))
        invq = rms_rep(lambda dk: qT[:, dk, :], "q", BF16)
        nc.vector.tensor_tensor(
            qT[:], qT[:], invq[:, None, :].to_broadcast((P, TD, T)), ALU.mult)
        sig = qT

        poolA.release()
        psA.release()

        # ---- per-(tchunk, c-tile) cumsum scans -> y = sig*kvcum/(wcum+eps),
        # in small f32 scratch so matmul2 can chase y tchunk by tchunk.
        # w-scans on DVE, wv-scans on GpSimd (concurrent engines). ----
        lastw = persist.tile([P, TD], F32, name="lastw")
        lastkv = persist.tile([P, TD], F32, name="lastkv")
        y = big_tile("y")
        for tci in range(NT):
            tsl = slice(tci * TC, (tci + 1) * TC)
            nc.scalar.activation(sig[:, :, tsl], qT[:, :, tsl], AF.Sigmoid)
            for ct in range(TD):
                wc = small.tile([P, TC], F32, name=f"wc_{tci}_{ct}",
                                tag="mid", bufs=5)
                init_w = (carry_eps[:, ct:ct + 1] if tci == 0
                          else lastw[:, ct:ct + 1])
                nc.vector.tensor_tensor_scan(
                    wc[:], w[:, ct, tsl], w[:, ct, tsl], init_w,
                    ALU.add, ALU.bypass)
                kv = small.tile([P, TC], F32, name=f"kv_{tci}_{ct}",
                                tag="mid", bufs=5)
                init_kv = (carry[:, TD + ct:TD + ct + 1] if tci == 0
                           else lastkv[:, ct:ct + 1])
                nc.vector.tensor_tensor_scan(
                    kv[:], wv[:, ct, tsl], wv[:, ct, tsl], init_kv,
                    ALU.add, ALU.bypass)
                if tci + 1 < NT:
                    nc.vector.tensor_copy(lastw[:, ct:ct + 1],
                                          wc[:, TC - 1:TC])
                    nc.vector.tensor_copy(lastkv[:, ct:ct + 1],
                                          kv[:, TC - 1:TC])
                rcp = small.tile([P, TC], F32, name=f"rcp_{tci}_{ct}",
                                 tag="mid", bufs=5)
                nc.vector.reciprocal_approx_fast(rcp[:], wc[:])
                kvy = small.tile([P, TC], F32, name=f"kvy_{tci}_{ct}",
                                 tag="mid", bufs=5)
                nc.vector.tensor_tensor(kvy[:], kv[:], rcp[:], ALU.mult)
                nc.vector.tensor_tensor(y[:, ct, tsl], kvy[:],
                                        sig[:, ct, tsl], ALU.mult)
        yT = y

        # ---- matmul2 (uv^T, h^T = u*silu(g)) + matmul3 (+residual) ----
        # h^T is held as two half tiles so it can recycle "big"-tag slots.
        FH = FU // 2
        poolB = tc.alloc_tile_pool(name="phaseB", bufs=1)
        psB = tc.alloc_tile_pool(name="psB", bufs=1, space="PSUM")
        for tci in range(NT):
            tsl = slice(tci * TC, (tci + 1) * TC)
            hT_halves = [big.tile([P, FH, TC], BF16, name=f"hT_{tci}_{i}",
                                  tag="big", bufs=4) for i in range(2)]

            def hT_slice(k, tt):
                return hT_halves[k // FH][:, k % FH, tt * P:(tt + 1) * P]

            for fj in range(FU):
                wu_t = poolB.tile([P, TD, P], BF16, name=f"wu_{tci}_{fj}",
                                  tag="ws", bufs=4)
                nc.sync.dma_start(wu_t[:], ws_v[:, :, fj * P:(fj + 1) * P])
                wg_t = poolB.tile([P, TD, P], BF16, name=f"wg_{tci}_{fj}",
                                  tag="ws", bufs=4)
                nc.sync.dma_start(
                    wg_t[:], ws_v[:, :, DFF + fj * P:DFF + (fj + 1) * P])
                psu = psB.tile([P, TC], F32, name=f"psu_{tci}_{fj}",
                               tag="mm2", bufs=4, space="PSUM")
                psg = psB.tile([P, TC], F32, name=f"psg_{tci}_{fj}",
                               tag="mm2", bufs=4, space="PSUM")
                for dk in range(TD):
                    nc.tensor.matmul(psu[:], wu_t[:, dk, :], yT[:, dk, tsl],
                                     start=(dk == 0), stop=(dk == TD - 1))
                for dk in range(TD):
                    nc.tensor.matmul(psg[:], wg_t[:, dk, :], yT[:, dk, tsl],
                                     start=(dk == 0), stop=(dk == TD - 1))
                sg = poolB.tile([P, TC], BF16, name=f"sg_{tci}_{fj}",
                                tag="sg", bufs=3)
                nc.scalar.activation(sg[:], psg[:], AF.Sigmoid)
                gsg = poolB.tile([P, TC], BF16, name=f"gsg_{tci}_{fj}",
                                 tag="sg", bufs=3)
                nc.vector.tensor_tensor(gsg[:], psg[:], sg[:], ALU.mult)
                nc.vector.tensor_tensor(hT_halves[fj // FH][:, fj % FH, :],
                                        psu[:], gsg[:], ALU.mult)

            n_tt = TC // P
            for dc in range(ND):
                dsl = slice(dc * DC, (dc + 1) * DC)
                ps3 = [psB.tile([P, DC], F32, name=f"ps3_{tci}_{dc}_{tt}",
                                tag="mm3", bufs=4, space="PSUM")
                       for tt in range(n_tt)]
                for kg in range(FU // KG):
                    wo_t = poolB.tile([P, KG, DC], BF16,
                                      name=f"wo_{tci}_{dc}_{kg}",
                                      tag="wo", bufs=2)
                    nc.sync.dma_start(
                        wo_t[:], wo_v[:, kg * KG:(kg + 1) * KG, dsl])
                    for tt in range(n_tt):
                        for kk in range(KG):
                            k = kg * KG + kk
                            nc.tensor.matmul(
                                ps3[tt][:], hT_slice(k, tt), wo_t[:, kk, :],
                                start=(k == 0), stop=(k == FU - 1))
                for tt in range(n_tt):
                    tt_g = tci * (TC // P) + tt
                    xr_t = poolB.tile([P, DC], F32,
                                      name=f"xr_{tci}_{dc}_{tt}",
                                      tag="xr", bufs=3)
                    nc.sync.dma_start(xr_t[:], xr_v[:, tt_g, dsl])
                    o_t = poolB.tile([P, DC], F32, name=f"o_{tci}_{dc}_{tt}",
                                     tag="ot", bufs=3)
                    nc.vector.tensor_tensor(o_t[:], ps3[tt][:], xr_t[:],
                                            ALU.add)
                    nc.sync.dma_start(out_v[:, tt_g, dsl], o_t[:])

        poolB.release()
        psB.release()
        small.release()
        big.release()
        persist.release()

    nc.compile()
    return nc


_NC_CACHE = {}


def _get_nc(B, S, D, DFF):
    key = (B, S, D, DFF)
    if key not in _NC_CACHE:
        _NC_CACHE[key] = build_nc(B, S, D, DFF)
    return _NC_CACHE[key]


def make_in_maps(x, w_qkv, w_swiglu, w_out):
    B, S, D = x.shape
    T = S // 2
    bf = ml_dtypes.bfloat16
    wq_T = w_qkv.T.astype(bf)
    ws_T = w_swiglu.T.astype(bf)
    wo_T = w_out.T.astype(bf)
    in_maps = []
    for c in range(N_CORES):
        b, h = divmod(c, 2)
        xc = x[b, h * T:(h + 1) * T]
        in_maps.append({
            "xT": xc.T.astype(bf),
            "xres": np.ascontiguousarray(xc, dtype=np.float32),
            "wqkvT": wq_T,
            "wsT": ws_T,
            "woT": wo_T,
            "mask": np.array([[1.0 - h, float(h)]], np.float32),
        })
    return in_maps


def assemble_out(results, B, S, D):
    T = S // 2
    out = np.empty((B, S, D), np.float32)
    for c in range(N_CORES):
        b, h = divmod(c, 2)
        out[b, h * T:(h + 1) * T] = results[c]["out"]
    return out


def kernel(x, w_qkv, w_swiglu, w_out):
    x = np.asarray(x, dtype=np.float32)
    w_qkv = np.asarray(w_qkv, dtype=np.float32)
    w_swiglu = np.asarray(w_swiglu, dtype=np.float32)
    w_out = np.asarray(w_out, dtype=np.float32)
    B, S, D = x.shape
    DFF = w_out.shape[1]
    nc = _get_nc(B, S, D, DFF)
    in_maps = make_in_maps(x, w_qkv, w_swiglu, w_out)
    res = run_bass_kernel_spmd(nc, in_maps, core_ids=list(range(N_CORES)))
    return assemble_out(res.results, B, S, D)
